# revision 63
# baseline (speedup 1.0000x reference)
"""Trainium2 Bass kernel for linear multi-head attention (elu+1 feature map).

Math (per batch n):
  q = x_q @ Wq.T ; k = x_k @ Wk.T ; v = (x_v @ Wv.T) / L
  Q = elu(q)+1 ; K = elu(k)+1
  KV[h] = K_h.T @ v_h              (D x D per head)
  Ksum  = sum_s K[s, :]            (E)
  S[l,h] = Q_h[l] . Ksum_h ;  W = L / (S + eps)
  msg[l, h*D+dv] = (Q_h[l] @ KV[h])[dv] * W[l,h]
  out = msg @ Wm.T

Sharding: B*L = 16384 rows split into 8 chunks of 2048 (each core gets half
of one batch's sequence). Only cross-core dependency: the KV/Ksum reduction
between the two cores sharing a batch -> pairwise AllReduce (f32, 66KB).

Final structure (218.9us baseline -> 115.9us):
  - inputs/weights cast to bf16 on the host: halves HBM traffic + SBUF and
    enables fast weight loads (FWL); PSUM accumulation stays f32.
  - all input DMA on the sync (SP) HWDGE queue, phase-A tensors first
    (one DMA per 512-row stripe across all 4 k-blocks), q-side after.
  - engine warmups at t=0: dummy matmuls trip the HAM clock gate to
    K=8/8 and a dummy Exp pulls in the ACT table load; a tiny warm-up
    AllGather absorbs the ~11.5us first-collective ncfw dispatch cost.
  - KV cross-product is group-local (K_g^T @ V_g, N=128) packed into a
    single PSUM bank; Ksum kept in partition layout via N=1 matmuls;
    2-tile software-pipeline skew hides the elu chain.
  - pairwise bf16 AllGather of the packed [kv|ksum] partials + one local
    DVE add (faster than AllReduce; an 8-rank group measured worse).
  - S computed directly in replicated layout ([128, CH] per group) by
    matmul with a block-masked Ksum operand -> no partition-broadcast
    DMA; W = 1/(S/L+eps/L) via ACT scale/bias + DVE fast reciprocal.
  - phase B: per-chunk S/msg matmuls with merges interleaved one chunk
    behind; last chunk emits all S matmuls first to shorten the tail.
"""

import numpy as np

B = 4
L = 4096
E = 512
H = 16
D = 32
P = 128
KT = E // P
NCORES = 8
R = (B * L) // NCORES
ST = R // P
NCHUNK = 4
CH = R // NCHUNK
EPS = 1e-6
CW = P + KT          # packed collective row: 128 kv cols + 4 ksum cols
CC = P * CW

_CACHE = {}
LAST_EXEC_NS = None
LAST_RESULTS = None


def _build():
    import concourse.bass as bass
    import concourse.mybir as mybir
    import concourse.tile as tile
    from concourse import bacc

    f32 = mybir.dt.float32
    bf16 = mybir.dt.bfloat16
    fp8 = mybir.dt.float8e4
    DR = mybir.MatmulPerfMode.DoubleRow
    AFT = mybir.ActivationFunctionType
    OP = mybir.AluOpType

    nc = bacc.Bacc("TRN2", target_bir_lowering=False, debug=False,
                   num_devices=NCORES)

    # q/k sides ride fp8e4: halves their HBM traffic and double-pumps
    # their projection matmuls (DoubleRow). v side and Wm stay bf16 --
    # fp8 there fails the accuracy gate (no normalizer to cancel errors).
    xq_d = nc.dram_tensor("xq", [E, R], fp8, kind="ExternalInput").ap()
    xk_d = nc.dram_tensor("xk", [E, R], fp8, kind="ExternalInput").ap()
    xv_d = nc.dram_tensor("xv", [E, R], bf16, kind="ExternalInput").ap()
    wq_d = nc.dram_tensor("wq", [E, E], fp8, kind="ExternalInput").ap()
    wk_d = nc.dram_tensor("wk", [E, E], fp8, kind="ExternalInput").ap()
    wv_d = nc.dram_tensor("wv", [E, E], bf16, kind="ExternalInput").ap()
    wm_d = nc.dram_tensor("wm", [E, E], bf16, kind="ExternalInput").ap()
    out_d = nc.dram_tensor("out", [R, E], bf16, kind="ExternalOutput").ap()

    RG = [[0, 1], [2, 3], [4, 5], [6, 7]]

    with tile.TileContext(nc) as tc:

        with tc.tile_pool(name="const", bufs=1) as const, \
             tc.tile_pool(name="xq_pool", bufs=1) as xq_pool, \
             tc.tile_pool(name="qt_pool", bufs=1) as qt_pool, \
             tc.tile_pool(name="dram", bufs=1, space="DRAM") as dram:

            # tiny warm-up AllGather FIRST, with no input deps (payload is
            # uninitialized DRAM, result unused): the ~23us first-collective
            # device BARRIER + ~11us ncfw dispatch start immediately and
            # finish while phase A streams.
            ccw_in = dram.tile([16], bf16)
            ccw_out = dram.tile([32], bf16)
            nc.gpsimd.collective_compute(
                "AllGather", mybir.AluOpType.bypass, replica_groups=RG,
                ins=[ccw_in[:].opt()], outs=[ccw_out[:].opt()])

            wq_sb = const.tile([P, KT, E], fp8)
            # ---- engine warmups: dummy matmuls push HAM to K=8/8 and a
            # dummy Exp pulls the ACT table load off the critical path,
            # all while the input DMA prefill streams.
            warm_sb = const.tile([P, E], bf16)
            nc.vector.memset(warm_sb[:], 0.0)
            warm_f = const.tile([P, 4], f32)
            nc.scalar.activation(warm_f[:], warm_sb[:, 0:4], AFT.Exp)
            wk_sb = const.tile([P, KT, E], fp8)
            wv_sb = const.tile([P, KT, E], bf16)
            wm_sb = const.tile([P, KT, E], bf16)

            # block-identity mask scaled by 1/L: maskI[k, p] =
            # (k//32 == p//32) / L, so the S matmul emits S/L directly.
            maskI_np = np.zeros((P, P), np.float32)
            for j in range(4):
                maskI_np[32 * j:32 * (j + 1), 32 * j:32 * (j + 1)] = 1.0 / L
            maskI_d = nc.inline_tensor(maskI_np, name="blk_ident")
            maskI_sb = const.tile([P, P], f32)
            nc.gpsimd.dma_start(maskI_sb[:], maskI_d.ap())

            cc_in = dram.tile([CC], bf16)
            cc_out2 = dram.tile([2 * CC], bf16)

            wv_r = wv_d.rearrange("(ko ki) n -> ki ko n", ki=P)
            wk_r = wk_d.rearrange("(ko ki) n -> ki ko n", ki=P)
            wq_r = wq_d.rearrange("(ko ki) n -> ki ko n", ki=P)
            wm_r = wm_d.rearrange("(ko ki) n -> ki ko n", ki=P)
            xv_r = xv_d.rearrange("(ko ki) n -> ki ko n", ki=P)
            xk_r = xk_d.rearrange("(ko ki) n -> ki ko n", ki=P)
            xq_r = xq_d.rearrange("(ko ki) n -> ki ko n", ki=P)

            # =================== Phase A: k/v/q proj + KV/Ksum ==============
            qt_sb = qt_pool.tile([P, KT, R], bf16)
            with tc.tile_pool(name="xkv_pool", bufs=1) as xkv_pool, \
                 tc.tile_pool(name="workA", bufs=4) as workA, \
                 tc.tile_pool(name="psA", bufs=4, space="PSUM") as psA, \
                 tc.tile_pool(name="psQA", bufs=1, space="PSUM") as psQA, \
                 tc.tile_pool(name="kvp", bufs=1, space="PSUM") as kvp:

                xk_sb = xkv_pool.tile([P, KT, R], fp8)
                xv_sb = xkv_pool.tile([P, KT, R], bf16)
                xq_sb = xq_pool.tile([P, KT, R], fp8)

                # group-local KV+Ksum accumulators. Orientation 1 (kv01/
                # kv23) carries an appended ones-column in V, so col 128 of
                # each group's output IS Ksum (no N=1 matmuls). Orientation
                # 2 (kvT, all 4 groups in ONE bank) is V^T@K = KV
                # transposed, whose diagonal blocks feed the per-head
                # U = KV @ Wm fusion in phase B. No matmul region crosses
                # a 2KB bank edge.
                kv01 = kvp.tile([P, 2, 256], f32)
                kv23 = kvp.tile([P, 2, 256], f32)
                kvT = kvp.tile([P, KT, P], f32)

                # dummy matmuls (into the later-cleared kv banks) keep the
                # PE busy + HAM warming while the first input stripes stream
                for w in range(8):
                    tgt = kv01 if w % 2 == 0 else kv23
                    nc.tensor.matmul(tgt[:, (w // 2) % 2, :],
                                     warm_sb[:, 0:P], warm_sb[:, 0:256],
                                     start=True, stop=True)

                # ---- input DMA: v-side on the sync (SP) HWDGE ring,
                # k-side on the scalar (ACT) ring so the two streams
                # prefill in parallel; first tile's rows split out.
                # q-side tensors follow on sync.
                nc.sync.dma_start(wv_sb[:], wv_r)
                nc.scalar.dma_start(wk_sb[:], wk_r)
                nc.sync.dma_start(xv_sb[:, :, 0:P], xv_r[:, :, 0:P])
                nc.scalar.dma_start(xk_sb[:, :, 0:P], xk_r[:, :, 0:P])
                nc.sync.dma_start(xv_sb[:, :, P:CH], xv_r[:, :, P:CH])
                # fp8 tensors need WIDE column chunks: a 512-col fp8 slice
                # is only a 512B DMA line (half-efficiency); 1024+ cols
                # restores >=1KB lines.
                nc.scalar.dma_start(xk_sb[:, :, P:2 * CH], xk_r[:, :, P:2 * CH])
                nc.scalar.dma_start(
                    xk_sb[:, :, 2 * CH:R], xk_r[:, :, 2 * CH:R])
                # q-side interleaves with v-side so q projections can run
                # inside the phase-A stripe loop (wq + first xq half early)
                nc.sync.dma_start(wq_sb[:], wq_r)
                nc.sync.dma_start(xq_sb[:, :, 0:2 * CH], xq_r[:, :, 0:2 * CH])
                nc.sync.dma_start(xv_sb[:, :, CH:2 * CH], xv_r[:, :, CH:2 * CH])
                nc.sync.dma_start(
                    xq_sb[:, :, 2 * CH:R], xq_r[:, :, 2 * CH:R])
                nc.sync.dma_start(xv_sb[:, :, 2 * CH:3 * CH],
                                  xv_r[:, :, 2 * CH:3 * CH])
                nc.sync.dma_start(xv_sb[:, :, 3 * CH:R],
                                  xv_r[:, :, 3 * CH:R])
                nc.sync.dma_start(wm_sb[:], wm_r)

                # software pipeline: KV(si-2) emitted between projections of
                # si so the PE never waits for the 2.5us elu chain.
                # One q projection no-iteration rides along with each
                # stripe: its matmuls use the PE's slack and its elu the
                # ACT/DVE slack, so qt is READY when phase A ends (the
                # collective floor hides the slightly longer stripes).
                def emit_q(i):
                    c, no = i // KT, i % KT
                    cs = slice(c * CH, (c + 1) * CH)
                    q_ps = psQA.tile([P, CH], f32, name="q_ps")
                    for t in range(KT // 2):
                        nc.tensor.matmul(
                            q_ps[:],
                            wq_sb[:, 2 * t:2 * t + 2, no * P:(no + 1) * P],
                            xq_sb[:, 2 * t:2 * t + 2, cs],
                            start=(t == 0), stop=(t == KT // 2 - 1),
                            perf_mode=DR)
                    tA = workA.tile([P, CH], f32, name="tAq", tag="tAq")
                    if no % 2 == 0:
                        nc.scalar.activation(tA[:], q_ps[:], AFT.Relu,
                                             scale=-1.0)
                    else:
                        nc.vector.tensor_scalar(
                            tA[:], q_ps[:], -1.0, 0.0, OP.mult, OP.max)
                    tB = workA.tile([P, CH], f32, name="tBq", tag="tBq")
                    nc.scalar.activation(tB[:], tA[:], AFT.Exp, scale=-1.0)
                    nc.vector.scalar_tensor_tensor(
                        qt_sb[:, no, cs], q_ps[:], 0.0, tB[:],
                        OP.max, OP.add)

                SKEW = 2
                kv_q = {}
                for si in range(ST + SKEW):
                    if si < ST:
                        sl = slice(si * P, (si + 1) * P)
                        v_ps = psA.tile([P, E], f32, name="v_ps", tag="proj")
                        for ko in range(KT):
                            nc.tensor.matmul(
                                v_ps[:], xv_sb[:, ko, sl], wv_sb[:, ko, :],
                                start=(ko == 0), stop=(ko == KT - 1))
                        v_sb = workA.tile([P, KT, P + 1], bf16, name="v_sb")
                        nc.scalar.copy(
                            v_sb[:, :, 0:P],
                            v_ps[:].rearrange("p (g f) -> p g f", g=KT))
                        nc.vector.memset(v_sb[:, :, P], 1.0)

                        k_ps = psA.tile([P, E], f32, name="k_ps", tag="proj")
                        for t in range(KT // 2):
                            nc.tensor.matmul(
                                k_ps[:], xk_sb[:, 2 * t:2 * t + 2, sl],
                                wk_sb[:, 2 * t:2 * t + 2, :],
                                start=(t == 0), stop=(t == KT // 2 - 1),
                                perf_mode=DR)
                        # elu(x)+1 = Exp(-Relu(-x)) + max(x,0)
                        # Relu step on DVE, Exp on ACT, combine on DVE.
                        tA = workA.tile([P, E], f32, name="tAk", tag="tAk")
                        nc.vector.tensor_scalar(
                            tA[:], k_ps[:], -1.0, 0.0, OP.mult, OP.max)
                        tB = workA.tile([P, E], f32, name="tBk", tag="tBk")
                        nc.scalar.activation(tB[:], tA[:], AFT.Exp,
                                             scale=-1.0)
                        k_sb = workA.tile([P, E], bf16, name="k_sb")
                        nc.vector.scalar_tensor_tensor(
                            k_sb[:], k_ps[:], 0.0, tB[:], OP.max, OP.add)
                        kv_q[si] = (k_sb, v_sb)
                    if si >= SKEW:
                        pk, pv = kv_q.pop(si - SKEW)
                        for g in range(KT):
                            gsl = slice(g * P, (g + 1) * P)
                            tgt = kv01 if g < 2 else kv23
                            # start=True clears has_written for the WHOLE
                            # bank: only the first write per bank sets it.
                            nc.tensor.matmul(
                                tgt[:, g % 2, 0:P + 1], pk[:, gsl],
                                pv[:, g, :],
                                start=(si == SKEW and g % 2 == 0),
                                stop=(si == ST + SKEW - 1))
                            nc.tensor.matmul(
                                kvT[:, g, :], pv[:, g, 0:P],
                                pk[:, gsl],
                                start=(si == SKEW and g == 0),
                                stop=(si == ST + SKEW - 1))
                    if 1 <= si <= ST:
                        emit_q(si - 1)

                # pack [kvT diag blocks | ksum] into one [P, 132] bf16
                # tile, then a single store for the collective.
                kvks_sb = workA.tile([P, CW], bf16, name="kvks_sb")
                kv_view = kvks_sb[:, 0:P].rearrange("p (g f) -> p g f", g=KT)
                for j in range(KT):
                    rs = slice(32 * j, 32 * (j + 1))
                    nc.vector.tensor_copy(
                        kv_view[rs, :, :], kvT[rs, :, rs])
                nc.vector.tensor_copy(kvks_sb[:, P:P + 2], kv01[:, :, P])
                nc.vector.tensor_copy(kvks_sb[:, P + 2:P + 4], kv23[:, :, P])
                nc.sync.dma_start(
                    cc_in[:].rearrange("(p f) -> p f", p=P), kvks_sb[:])

            # ============ pairwise AllGather (reduce locally after) =========
            # AG is a pure copy (no CCE reduce): measured 9.6us vs 16.9us
            # for a pairwise AllReduce of half the payload; 8-rank groups
            # are worse still (all-rank barrier on the slowest core).
            nc.gpsimd.collective_compute(
                "AllGather", mybir.AluOpType.bypass, replica_groups=RG,
                ins=[cc_in[:].opt()], outs=[cc_out2[:].opt()])

            # ============ bridge: phase A ends well before the AllGather
            # lands; dummy matmuls keep HAM at K=8/8 across the idle window
            with tc.tile_pool(name="psQ", bufs=1, space="PSUM") as psQ:
                bridge_ps = psQ.tile([P, CH], f32, name="q_ps")
                for _ in range(12):
                    nc.tensor.matmul(bridge_ps[:], warm_sb[:, 0:P],
                                     warm_sb[:], start=True, stop=True)

            # ---- gathered partials -> one local add -> kv / ks
            two = const.tile([P, 2, CW], bf16)
            nc.sync.dma_start(
                two[:], cc_out2[:].rearrange("(r p f) -> p r f", r=2, p=P))
            red = const.tile([P, CW], bf16)
            nc.vector.tensor_tensor(
                red[:], two[:, 0, :], two[:, 1, :], OP.add)
            ks_f32 = const.tile([P, KT], f32)
            nc.vector.tensor_copy(ks_f32[:], red[:, P:CW])
            # bd_rep[:, g, p] = Ksum[128g + k] if k//32 == p//32 else 0
            bd_rep = const.tile([P, KT, P], bf16)
            for g in range(KT):
                nc.vector.tensor_tensor(
                    bd_rep[:, g, :], maskI_sb[:],
                    ks_f32[:, g, None].to_broadcast((P, P)), OP.mult)

            with tc.tile_pool(name="workB", bufs=4) as workB, \
                 tc.tile_pool(name="spool", bufs=2, space="PSUM") as spool, \
                 tc.tile_pool(name="opool", bufs=1, space="PSUM") as opool:

                # U_g = KV_g @ Wm_g per head (associativity: out =
                # ((Q*W)@KV)@Wm = (Q*W)@(KV@Wm)), computed ONCE from the
                # gathered KV^T diag blocks -- this replaces the per-chunk
                # msg matmuls entirely and fuses msg+merge into one GEMM.
                u_sb = const.tile([P, KT, E], bf16)
                for g in range(KT):
                    u_ps = spool.tile([P, E], f32, name="s_ps")
                    for j in range(KT):
                        rs = slice(32 * j, 32 * (j + 1))
                        nc.tensor.matmul(
                            u_ps[rs, :],
                            red[rs, 32 * g:32 * (g + 1)],
                            wm_sb[rs, g, :],
                            start=True, stop=True,
                            tile_position=(32 * j, 32 * j))
                    nc.scalar.copy(u_sb[:, g, :], u_ps[:])

                # S / recip / qw-mult run 1024 wide (two output chunks per
                # op) to amortize the ~400ns fixed cost of each DVE op;
                # the merge accumulates per-g into 4 held PSUM banks at
                # 512 granularity, two steps behind the qw chain.
                SW = 2 * CH
                o_tiles = {}
                qws = {}

                def emit_sw(c2, g):
                    s_ps = spool.tile([P, SW], f32, name="s_ps")
                    # HAM filler: a dummy matmul keeps PE activity dense
                    # enough that the clock stays K=8/8; the real S
                    # matmul's start=True overwrites it.
                    nc.tensor.matmul(
                        s_ps[:, 0:256], warm_sb[:, 0:P],
                        warm_sb[:, 0:256], start=True, stop=True)
                    # replicated S/L (maskI carries 1/L):
                    # s_ps[p, l] = S[l, 4g + p//32] / L
                    for h in range(2):
                        hs = slice(h * CH, (h + 1) * CH)
                        nc.tensor.matmul(
                            s_ps[:, hs], bd_rep[:, g, :],
                            qt_sb[:, g, slice(c2 * SW + h * CH,
                                              c2 * SW + (h + 1) * CH)],
                            start=True, stop=True)
                    # W = L/S via DVE fast recip; S/L ~ 37 >> eps/L, the
                    # eps guard is numerically irrelevant for these inputs
                    w_r = workB.tile([P, SW], f32, name="w_r",
                                     tag=f"w_r{g % 2}")
                    nc.vector.reciprocal_approx_fast(w_r[:], s_ps[:])
                    qw = workB.tile([P, SW], bf16, name="qw_sb")
                    nc.vector.tensor_tensor(
                        qw[:], qt_sb[:, g, slice(c2 * SW, (c2 + 1) * SW)],
                        w_r[:], OP.mult)
                    qws[(c2, g)] = qw

                def do_merge(c, g):
                    qw = qws[(c // 2, g)]
                    off = (c % 2) * CH
                    if g == 0:
                        o_tiles[c] = [opool.tile([P, E], f32, name=f"o{lt}")
                                      for lt in range(CH // P)]
                    for lt in range(CH // P):
                        nc.tensor.matmul(
                            o_tiles[c][lt][:],
                            qw[:, off + lt * P:off + (lt + 1) * P],
                            u_sb[:, g, :],
                            start=(g == 0), stop=(g == KT - 1))
                    if g == KT - 1:
                        # one [P, 4, E] staging tile -> a single out DMA
                        # per chunk (DMA issue on sync costs ~0.65us each).
                        # DVE helps copy only on the last chunk (it paces
                        # the chunks before; it is idle at the tail).
                        o_sb = workB.tile([P, CH // P, E], bf16,
                                          name="o_sb")
                        for lt in range(CH // P):
                            if c == NCHUNK - 1 and lt % 2 == 1:
                                nc.vector.tensor_copy(
                                    o_sb[:, lt, :], o_tiles[c][lt][:])
                            else:
                                nc.scalar.copy(
                                    o_sb[:, lt, :], o_tiles[c][lt][:])
                        nc.sync.dma_start(
                            out_d[c * CH:(c + 1) * CH, :].rearrange(
                                "(lt p) n -> p lt n", p=P),
                            o_sb[:])

                # three-behind merge: the 1024-wide S -> recip -> mult
                # chain (~3.5us) is covered by three steps of PE work, so
                # the PE never stalls waiting on the DVE.
                pending = []
                for c in range(NCHUNK):
                    for g in range(KT):
                        if c % 2 == 0:
                            emit_sw(c // 2, g)
                        if len(pending) >= 3:
                            do_merge(*pending.pop(0))
                        pending.append((c, g))
                for pd in pending:
                    do_merge(*pd)

    nc.compile()
    return nc


def _get_nc():
    if "nc" not in _CACHE:
        _CACHE["nc"] = _build()
    return _CACHE["nc"]


def kernel(query, key, value, Wq, Wk, Wv, Wm):
    global LAST_EXEC_NS, LAST_RESULTS
    import os
    import ml_dtypes
    from concourse.bass_utils import run_bass_kernel_spmd

    bf = ml_dtypes.bfloat16
    f8 = ml_dtypes.float8_e4m3
    query = np.asarray(query, dtype=np.float32)
    key = np.asarray(key, dtype=np.float32)
    value = np.asarray(value, dtype=np.float32)
    wq_t = np.ascontiguousarray(np.asarray(Wq, np.float32).T).astype(f8)
    wk_t = np.ascontiguousarray(np.asarray(Wk, np.float32).T).astype(f8)
    wv_t = np.ascontiguousarray(
        np.asarray(Wv, np.float32).T / L).astype(bf)
    wm_t = np.ascontiguousarray(np.asarray(Wm, np.float32).T).astype(bf)

    in_maps = []
    for c in range(NCORES):
        b, half = c // 2, c % 2
        rs = slice(half * R, (half + 1) * R)
        in_maps.append({
            "xq": np.ascontiguousarray(query[b, rs, :].T).astype(f8),
            "xk": np.ascontiguousarray(key[b, rs, :].T).astype(f8),
            "xv": np.ascontiguousarray(value[b, rs, :].T).astype(bf),
            "wq": wq_t, "wk": wk_t, "wv": wv_t, "wm": wm_t,
        })

    nc = _get_nc()
    trace = bool(int(os.environ.get("KERNEL_TRACE", "0")))
    res = run_bass_kernel_spmd(nc, in_maps, core_ids=list(range(NCORES)),
                               trace=trace)
    LAST_EXEC_NS = res.exec_time_ns
    LAST_RESULTS = res

    out = np.empty((B, L, E), dtype=np.float32)
    for c in range(NCORES):
        b, half = c // 2, c % 2
        out[b, half * R:(half + 1) * R, :] = \
            res.results[c]["out"].astype(np.float32)
    return out



# revision 80
# speedup vs baseline: 1.1696x; 1.1696x over previous
"""Trainium2 Bass kernel for linear multi-head attention (elu+1 feature map).

Math (per batch n):
  q = x_q @ Wq.T ; k = x_k @ Wk.T ; v = (x_v @ Wv.T) / L
  Q = elu(q)+1 ; K = elu(k)+1
  KV[h] = K_h.T @ v_h              (D x D per head)
  Ksum  = sum_s K[s, :]            (E)
  S[l,h] = Q_h[l] . Ksum_h ;  W = L / (S + eps)
  out[l] = sum_h W[l,h] * (Q_h[l] @ KV[h]) @ Wm_h.T
         = ((Q*W) @ U) with U_h = KV_h @ Wm_h.T   (associativity)

Sharding: B*L = 16384 rows split into 8 chunks of 2048 (each core gets half
of one batch's sequence). Only cross-core dependency: the KV/Ksum reduction
between the two cores sharing a batch -> pairwise AllGather (bf16, 34KB)
plus one local add.

Structure (baseline 115us -> ~104-108us measured, run variance +-5us from
collective launch skew):
  - q/k sides in fp8e4 (DoubleRow double-pumped projections + half DMA);
    v side, Wm, and all accumulation stay bf16/f32 -- fp8 there fails the
    2e-2 gate (deterministic inputs: rel err 1.28e-2 vs 5.3e-3 all-bf16).
    fp8 DMA slices kept >=1024 cols so per-partition lines stay >=1KB.
  - engine warmups at t=0: dummy matmuls trip the HAM clock gate to
    K=8/8 and a dummy Exp pulls in the ACT table load; a tiny warm-up
    AllGather with NO input deps fires immediately and absorbs the
    ~20us first-collective device barrier + ~11us ncfw dispatch.
  - phase A: KV accumulated BOTH orientations (K^T@V with a ones-column
    appended to V so col 128 is Ksum -- no N=1 matmuls; and V^T@K whose
    diag blocks are the KV^T the U-fusion needs); 2-stripe software
    pipeline skew hides the elu chain.
  - pairwise bf16 AllGather of packed [kvT diag|ksum] + local DVE add
    (pairwise AllReduce measured 2x slower; 8-rank groups worse still).
  - q projection + elu in 1024-wide chunks overlap the AllGather window;
    elu = 3 elementwise passes split across ACT/DVE paces this span.
  - phase B fuses msg+merge into (Q*W) @ U: U = KV@Wm computed once (16
    tile-position matmuls), S replicated via block-masked Ksum matmul
    (maskI carries 1/L), W = recip(S/L) on DVE, qw mult 1024-wide, merge
    accumulated per-g into 4 held PSUM banks two steps behind the qw
    chain; dummy matmuls keep HAM at K=8/8 (phase B is DVE-paced).
  - bf16 output staged [P,4,E] -> one DMA per 512-row chunk.
"""

import numpy as np

B = 4
L = 4096
E = 512
H = 16
D = 32
P = 128
KT = E // P
NCORES = 8
R = (B * L) // NCORES
ST = R // P
NCHUNK = 4
CH = R // NCHUNK
EPS = 1e-6
CW = P + KT          # packed collective row: 128 kv cols + 4 ksum cols
CC = P * CW

_CACHE = {}
LAST_EXEC_NS = None
LAST_RESULTS = None


def _build():
    import concourse.bass as bass
    import concourse.mybir as mybir
    import concourse.tile as tile
    from concourse import bacc

    f32 = mybir.dt.float32
    bf16 = mybir.dt.bfloat16
    fp8 = mybir.dt.float8e4
    DR = mybir.MatmulPerfMode.DoubleRow
    AFT = mybir.ActivationFunctionType
    OP = mybir.AluOpType

    nc = bacc.Bacc("TRN2", target_bir_lowering=False, debug=False,
                   num_devices=NCORES)

    # q/k sides ride fp8e4: halves their HBM traffic and double-pumps
    # their projection matmuls (DoubleRow). v side and Wm stay bf16 --
    # fp8 there fails the accuracy gate (no normalizer to cancel errors).
    xq_d = nc.dram_tensor("xq", [E, R], fp8, kind="ExternalInput").ap()
    xk_d = nc.dram_tensor("xk", [E, R], fp8, kind="ExternalInput").ap()
    xv_d = nc.dram_tensor("xv", [E, R], bf16, kind="ExternalInput").ap()
    wq_d = nc.dram_tensor("wq", [E, E], fp8, kind="ExternalInput").ap()
    wk_d = nc.dram_tensor("wk", [E, E], fp8, kind="ExternalInput").ap()
    wv_d = nc.dram_tensor("wv", [E, E], bf16, kind="ExternalInput").ap()
    wm_d = nc.dram_tensor("wm", [E, E], bf16, kind="ExternalInput").ap()
    out_d = nc.dram_tensor("out", [R, E], bf16, kind="ExternalOutput").ap()

    RG = [[0, 1], [2, 3], [4, 5], [6, 7]]

    with tile.TileContext(nc) as tc:

        with tc.tile_pool(name="const", bufs=1) as const, \
             tc.tile_pool(name="xq_pool", bufs=1) as xq_pool, \
             tc.tile_pool(name="qt_pool", bufs=1) as qt_pool, \
             tc.tile_pool(name="dram", bufs=1, space="DRAM") as dram:

            # tiny warm-up AllGather FIRST, with no input deps (payload is
            # uninitialized DRAM, result unused): the ~23us first-collective
            # device BARRIER + ~11us ncfw dispatch start immediately and
            # finish while phase A streams.
            ccw_in = dram.tile([16], bf16)
            ccw_out = dram.tile([32], bf16)
            nc.gpsimd.collective_compute(
                "AllGather", mybir.AluOpType.bypass, replica_groups=RG,
                ins=[ccw_in[:].opt()], outs=[ccw_out[:].opt()])

            wq_sb = const.tile([P, KT, E], fp8)
            # ---- engine warmups: dummy matmuls push HAM to K=8/8 and a
            # dummy Exp pulls the ACT table load off the critical path,
            # all while the input DMA prefill streams.
            warm_sb = const.tile([P, E], bf16)
            nc.vector.memset(warm_sb[:], 0.0)
            warm_f = const.tile([P, 4], f32)
            nc.scalar.activation(warm_f[:], warm_sb[:, 0:4], AFT.Exp)
            wk_sb = const.tile([P, KT, E], fp8)
            wv_sb = const.tile([P, KT, E], bf16)
            wm_sb = const.tile([P, KT, E], bf16)

            # block-identity mask scaled by 1/L: maskI[k, p] =
            # (k//32 == p//32) / L, so the S matmul emits S/L directly.
            maskI_np = np.zeros((P, P), np.float32)
            for j in range(4):
                maskI_np[32 * j:32 * (j + 1), 32 * j:32 * (j + 1)] = 1.0 / L
            maskI_d = nc.inline_tensor(maskI_np, name="blk_ident")
            maskI_sb = const.tile([P, P], f32)
            nc.gpsimd.dma_start(maskI_sb[:], maskI_d.ap())

            cc_in = dram.tile([CC], bf16)
            cc_out2 = dram.tile([2 * CC], bf16)

            wv_r = wv_d.rearrange("(ko ki) n -> ki ko n", ki=P)
            wk_r = wk_d.rearrange("(ko ki) n -> ki ko n", ki=P)
            wq_r = wq_d.rearrange("(ko ki) n -> ki ko n", ki=P)
            wm_r = wm_d.rearrange("(ko ki) n -> ki ko n", ki=P)
            xv_r = xv_d.rearrange("(ko ki) n -> ki ko n", ki=P)
            xk_r = xk_d.rearrange("(ko ki) n -> ki ko n", ki=P)
            xq_r = xq_d.rearrange("(ko ki) n -> ki ko n", ki=P)

            # =================== Phase A: k/v proj + KV/Ksum ===============
            with tc.tile_pool(name="xkv_pool", bufs=1) as xkv_pool, \
                 tc.tile_pool(name="workA", bufs=4) as workA, \
                 tc.tile_pool(name="psA", bufs=4, space="PSUM") as psA, \
                 tc.tile_pool(name="kvp", bufs=1, space="PSUM") as kvp:

                xk_sb = xkv_pool.tile([P, KT, R], fp8)
                xv_sb = xkv_pool.tile([P, KT, R], bf16)
                xq_sb = xq_pool.tile([P, KT, R], fp8)

                # group-local KV+Ksum accumulators. Orientation 1 (kv01/
                # kv23) carries an appended ones-column in V, so col 128 of
                # each group's output IS Ksum (no N=1 matmuls). Orientation
                # 2 (kvT, all 4 groups in ONE bank) is V^T@K = KV
                # transposed, whose diagonal blocks feed the per-head
                # U = KV @ Wm fusion in phase B. No matmul region crosses
                # a 2KB bank edge.
                kv01 = kvp.tile([P, 2, 256], f32)
                kv23 = kvp.tile([P, 2, 256], f32)
                kvT = kvp.tile([P, KT, P], f32)

                # dummy matmuls (into the later-cleared kv banks) keep the
                # PE busy + HAM warming while the first input stripes stream
                for w in range(8):
                    tgt = kv01 if w % 2 == 0 else kv23
                    nc.tensor.matmul(tgt[:, (w // 2) % 2, :],
                                     warm_sb[:, 0:P], warm_sb[:, 0:256],
                                     start=True, stop=True)

                # ---- input DMA: v-side on the sync (SP) HWDGE ring,
                # k-side on the scalar (ACT) ring so the two streams
                # prefill in parallel; first tile's rows split out.
                # q-side tensors follow on sync.
                nc.sync.dma_start(wv_sb[:], wv_r)
                nc.scalar.dma_start(wk_sb[:], wk_r)
                nc.sync.dma_start(xv_sb[:, :, 0:P], xv_r[:, :, 0:P])
                nc.scalar.dma_start(xk_sb[:, :, 0:P], xk_r[:, :, 0:P])
                nc.sync.dma_start(xv_sb[:, :, P:CH], xv_r[:, :, P:CH])
                # fp8 tensors need WIDE column chunks: a 512-col fp8 slice
                # is only a 512B DMA line (half-efficiency); 1024+ cols
                # restores >=1KB lines.
                nc.scalar.dma_start(xk_sb[:, :, P:2 * CH], xk_r[:, :, P:2 * CH])
                nc.scalar.dma_start(
                    xk_sb[:, :, 2 * CH:R], xk_r[:, :, 2 * CH:R])
                for sc in range(1, NCHUNK):
                    cs = slice(sc * CH, (sc + 1) * CH)
                    nc.sync.dma_start(xv_sb[:, :, cs], xv_r[:, :, cs])
                nc.sync.dma_start(wq_sb[:], wq_r)
                for c2 in range(NCHUNK // 2):
                    cs = slice(c2 * 2 * CH, (c2 + 1) * 2 * CH)
                    nc.sync.dma_start(xq_sb[:, :, cs], xq_r[:, :, cs])
                nc.sync.dma_start(wm_sb[:], wm_r)

                # software pipeline: KV(si-2) emitted between projections of
                # si so the PE never waits for the 2.5us elu chain
                SKEW = 2
                kv_q = {}
                for si in range(ST + SKEW):
                    if si < ST:
                        sl = slice(si * P, (si + 1) * P)
                        v_ps = psA.tile([P, E], f32, name="v_ps", tag="proj")
                        for ko in range(KT):
                            nc.tensor.matmul(
                                v_ps[:], xv_sb[:, ko, sl], wv_sb[:, ko, :],
                                start=(ko == 0), stop=(ko == KT - 1))
                        v_sb = workA.tile([P, KT, P + 1], bf16, name="v_sb")
                        nc.scalar.copy(
                            v_sb[:, :, 0:P],
                            v_ps[:].rearrange("p (g f) -> p g f", g=KT))
                        nc.vector.memset(v_sb[:, :, P], 1.0)

                        k_ps = psA.tile([P, E], f32, name="k_ps", tag="proj")
                        for t in range(KT // 2):
                            nc.tensor.matmul(
                                k_ps[:], xk_sb[:, 2 * t:2 * t + 2, sl],
                                wk_sb[:, 2 * t:2 * t + 2, :],
                                start=(t == 0), stop=(t == KT // 2 - 1),
                                perf_mode=DR)
                        # elu(x)+1 = Exp(-Relu(-x)) + max(x,0)
                        # Relu step on DVE, Exp on ACT, combine on DVE.
                        tA = workA.tile([P, E], f32, name="tAk", tag="tAk")
                        nc.vector.tensor_scalar(
                            tA[:], k_ps[:], -1.0, 0.0, OP.mult, OP.max)
                        tB = workA.tile([P, E], f32, name="tBk", tag="tBk")
                        nc.scalar.activation(tB[:], tA[:], AFT.Exp,
                                             scale=-1.0)
                        k_sb = workA.tile([P, E], bf16, name="k_sb")
                        nc.vector.scalar_tensor_tensor(
                            k_sb[:], k_ps[:], 0.0, tB[:], OP.max, OP.add)
                        kv_q[si] = (k_sb, v_sb)
                    if si >= SKEW:
                        pk, pv = kv_q.pop(si - SKEW)
                        for g in range(KT):
                            gsl = slice(g * P, (g + 1) * P)
                            tgt = kv01 if g < 2 else kv23
                            # start=True clears has_written for the WHOLE
                            # bank: only the first write per bank sets it.
                            nc.tensor.matmul(
                                tgt[:, g % 2, 0:P + 1], pk[:, gsl],
                                pv[:, g, :],
                                start=(si == SKEW and g % 2 == 0),
                                stop=(si == ST + SKEW - 1))
                            nc.tensor.matmul(
                                kvT[:, g, :], pv[:, g, 0:P],
                                pk[:, gsl],
                                start=(si == SKEW and g == 0),
                                stop=(si == ST + SKEW - 1))

                # pack [kvT diag blocks | ksum] into one [P, 132] bf16
                # tile, then a single store for the collective.
                kvks_sb = workA.tile([P, CW], bf16, name="kvks_sb")
                kv_view = kvks_sb[:, 0:P].rearrange("p (g f) -> p g f", g=KT)
                for j in range(KT):
                    rs = slice(32 * j, 32 * (j + 1))
                    nc.vector.tensor_copy(
                        kv_view[rs, :, :], kvT[rs, :, rs])
                nc.vector.tensor_copy(kvks_sb[:, P:P + 2], kv01[:, :, P])
                nc.vector.tensor_copy(kvks_sb[:, P + 2:P + 4], kv23[:, :, P])
                nc.sync.dma_start(
                    cc_in[:].rearrange("(p f) -> p f", p=P), kvks_sb[:])

            # ============ pairwise AllGather (reduce locally after) =========
            # AG is a pure copy (no CCE reduce): measured 9.6us vs 16.9us
            # for a pairwise AllReduce of half the payload; 8-rank groups
            # are worse still (all-rank barrier on the slowest core).
            nc.gpsimd.collective_compute(
                "AllGather", mybir.AluOpType.bypass, replica_groups=RG,
                ins=[cc_in[:].opt()], outs=[cc_out2[:].opt()])

            # =================== q projection + elu (overlaps AllGather) ====
            qt_sb = qt_pool.tile([P, KT, R], bf16)
            QW = 2 * CH  # 1024-wide q chunks amortize elu fixed op costs
            with tc.tile_pool(name="workQ", bufs=3) as workQ, \
                 tc.tile_pool(name="psQ", bufs=3, space="PSUM") as psQ:
                for c2 in range(NCHUNK // 2):
                    cs = slice(c2 * QW, (c2 + 1) * QW)
                    for no in range(KT):
                        q_ps = psQ.tile([P, QW], f32, name="q_ps")
                        for h in range(2):
                            hs = slice(h * CH, (h + 1) * CH)
                            xs = slice(c2 * QW + h * CH,
                                       c2 * QW + (h + 1) * CH)
                            for t in range(KT // 2):
                                nc.tensor.matmul(
                                    q_ps[:, hs],
                                    wq_sb[:, 2 * t:2 * t + 2,
                                          no * P:(no + 1) * P],
                                    xq_sb[:, 2 * t:2 * t + 2, xs],
                                    start=(t == 0),
                                    stop=(t == KT // 2 - 1),
                                    perf_mode=DR)
                        # alternate the Relu step ACT/DVE so neither engine
                        # paces the elu chain during the AllGather window
                        tA = workQ.tile([P, QW], f32, name="tAq", tag="tAq")
                        if no % 2 == 0:
                            nc.scalar.activation(tA[:], q_ps[:], AFT.Relu,
                                                 scale=-1.0)
                        else:
                            nc.vector.tensor_scalar(
                                tA[:], q_ps[:], -1.0, 0.0, OP.mult, OP.max)
                        tB = workQ.tile([P, QW], f32, name="tBq", tag="tBq")
                        nc.scalar.activation(tB[:], tA[:], AFT.Exp,
                                             scale=-1.0)
                        nc.vector.scalar_tensor_tensor(
                            qt_sb[:, no, cs], q_ps[:], 0.0, tB[:],
                            OP.max, OP.add)
                # small bridge of dummy matmuls: pads any PE-idle window
                # while the AllGather completes so HAM stays at K=8/8
                bridge_ps = psQ.tile([P, CH], f32, name="q_ps")
                for _ in range(8):
                    nc.tensor.matmul(bridge_ps[:], warm_sb[:, 0:P],
                                     warm_sb[:], start=True, stop=True)

            # ---- gathered partials -> one local add -> kv / ks
            two = const.tile([P, 2, CW], bf16)
            nc.sync.dma_start(
                two[:], cc_out2[:].rearrange("(r p f) -> p r f", r=2, p=P))
            red = const.tile([P, CW], bf16)
            nc.vector.tensor_tensor(
                red[:], two[:, 0, :], two[:, 1, :], OP.add)
            ks_f32 = const.tile([P, KT], f32)
            nc.vector.tensor_copy(ks_f32[:], red[:, P:CW])
            # bd_rep[:, g, p] = Ksum[128g + k] if k//32 == p//32 else 0
            bd_rep = const.tile([P, KT, P], bf16)
            for g in range(KT):
                nc.vector.tensor_tensor(
                    bd_rep[:, g, :], maskI_sb[:],
                    ks_f32[:, g, None].to_broadcast((P, P)), OP.mult)

            with tc.tile_pool(name="workB", bufs=4) as workB, \
                 tc.tile_pool(name="spool", bufs=2, space="PSUM") as spool, \
                 tc.tile_pool(name="opool", bufs=1, space="PSUM") as opool:

                # U_g = KV_g @ Wm_g per head (associativity: out =
                # ((Q*W)@KV)@Wm = (Q*W)@(KV@Wm)), computed ONCE from the
                # gathered KV^T diag blocks -- this replaces the per-chunk
                # msg matmuls entirely and fuses msg+merge into one GEMM.
                u_sb = const.tile([P, KT, E], bf16)
                for g in range(KT):
                    u_ps = spool.tile([P, E], f32, name="s_ps")
                    for j in range(KT):
                        rs = slice(32 * j, 32 * (j + 1))
                        nc.tensor.matmul(
                            u_ps[rs, :],
                            red[rs, 32 * g:32 * (g + 1)],
                            wm_sb[rs, g, :],
                            start=True, stop=True,
                            tile_position=(32 * j, 32 * j))
                    nc.scalar.copy(u_sb[:, g, :], u_ps[:])

                # S / recip / qw-mult run 1024 wide (two output chunks per
                # op) to amortize the ~400ns fixed cost of each DVE op;
                # the merge accumulates per-g into 4 held PSUM banks at
                # 512 granularity, two steps behind the qw chain.
                SW = 2 * CH
                o_tiles = {}
                qws = {}

                def emit_sw(c2, g):
                    s_ps = spool.tile([P, SW], f32, name="s_ps")
                    # HAM filler: a dummy matmul keeps PE activity dense
                    # enough that the clock stays K=8/8; the real S
                    # matmul's start=True overwrites it.
                    nc.tensor.matmul(
                        s_ps[:, 0:256], warm_sb[:, 0:P],
                        warm_sb[:, 0:256], start=True, stop=True)
                    # replicated S/L (maskI carries 1/L):
                    # s_ps[p, l] = S[l, 4g + p//32] / L
                    for h in range(2):
                        hs = slice(h * CH, (h + 1) * CH)
                        nc.tensor.matmul(
                            s_ps[:, hs], bd_rep[:, g, :],
                            qt_sb[:, g, slice(c2 * SW + h * CH,
                                              c2 * SW + (h + 1) * CH)],
                            start=True, stop=True)
                    # W = L/S via DVE fast recip; S/L ~ 37 >> eps/L, the
                    # eps guard is numerically irrelevant for these inputs
                    w_r = workB.tile([P, SW], f32, name="w_r",
                                     tag=f"w_r{g % 2}")
                    nc.vector.reciprocal_approx_fast(w_r[:], s_ps[:])
                    qw = workB.tile([P, SW], bf16, name="qw_sb")
                    nc.vector.tensor_tensor(
                        qw[:], qt_sb[:, g, slice(c2 * SW, (c2 + 1) * SW)],
                        w_r[:], OP.mult)
                    qws[(c2, g)] = qw

                def do_merge(c, g):
                    qw = qws[(c // 2, g)]
                    off = (c % 2) * CH
                    if g == 0:
                        o_tiles[c] = [opool.tile([P, E], f32, name=f"o{lt}")
                                      for lt in range(CH // P)]
                    for lt in range(CH // P):
                        nc.tensor.matmul(
                            o_tiles[c][lt][:],
                            qw[:, off + lt * P:off + (lt + 1) * P],
                            u_sb[:, g, :],
                            start=(g == 0), stop=(g == KT - 1))
                    if g == KT - 1:
                        # one [P, 4, E] staging tile -> a single out DMA
                        # per chunk (DMA issue on sync costs ~0.65us each).
                        # DVE helps copy only on the last chunk (it paces
                        # the chunks before; it is idle at the tail).
                        o_sb = workB.tile([P, CH // P, E], bf16,
                                          name="o_sb")
                        for lt in range(CH // P):
                            if c == NCHUNK - 1 and lt % 2 == 1:
                                nc.vector.tensor_copy(
                                    o_sb[:, lt, :], o_tiles[c][lt][:])
                            else:
                                nc.scalar.copy(
                                    o_sb[:, lt, :], o_tiles[c][lt][:])
                        nc.sync.dma_start(
                            out_d[c * CH:(c + 1) * CH, :].rearrange(
                                "(lt p) n -> p lt n", p=P),
                            o_sb[:])

                # two-behind merge: the 1024-wide S -> recip -> mult chain
                # is covered by the PE's S+merge work of two steps, so the
                # PE never stalls waiting on the DVE.
                pending = []
                for c in range(NCHUNK):
                    for g in range(KT):
                        if c % 2 == 0:
                            emit_sw(c // 2, g)
                        if len(pending) >= 2:
                            do_merge(*pending.pop(0))
                        pending.append((c, g))
                for pd in pending:
                    do_merge(*pd)

    nc.compile()
    return nc


def _get_nc():
    if "nc" not in _CACHE:
        _CACHE["nc"] = _build()
    return _CACHE["nc"]


def kernel(query, key, value, Wq, Wk, Wv, Wm):
    global LAST_EXEC_NS, LAST_RESULTS
    import os
    import ml_dtypes
    from concourse.bass_utils import run_bass_kernel_spmd

    bf = ml_dtypes.bfloat16
    f8 = ml_dtypes.float8_e4m3
    query = np.asarray(query, dtype=np.float32)
    key = np.asarray(key, dtype=np.float32)
    value = np.asarray(value, dtype=np.float32)
    wq_t = np.ascontiguousarray(np.asarray(Wq, np.float32).T).astype(f8)
    wk_t = np.ascontiguousarray(np.asarray(Wk, np.float32).T).astype(f8)
    wv_t = np.ascontiguousarray(
        np.asarray(Wv, np.float32).T / L).astype(bf)
    wm_t = np.ascontiguousarray(np.asarray(Wm, np.float32).T).astype(bf)

    in_maps = []
    for c in range(NCORES):
        b, half = c // 2, c % 2
        rs = slice(half * R, (half + 1) * R)
        in_maps.append({
            "xq": np.ascontiguousarray(query[b, rs, :].T).astype(f8),
            "xk": np.ascontiguousarray(key[b, rs, :].T).astype(f8),
            "xv": np.ascontiguousarray(value[b, rs, :].T).astype(bf),
            "wq": wq_t, "wk": wk_t, "wv": wv_t, "wm": wm_t,
        })

    nc = _get_nc()
    trace = bool(int(os.environ.get("KERNEL_TRACE", "0")))
    res = run_bass_kernel_spmd(nc, in_maps, core_ids=list(range(NCORES)),
                               trace=trace)
    LAST_EXEC_NS = res.exec_time_ns
    LAST_RESULTS = res

    out = np.empty((B, L, E), dtype=np.float32)
    for c in range(NCORES):
        b, half = c // 2, c % 2
        out[b, half * R:(half + 1) * R, :] = \
            res.results[c]["out"].astype(np.float32)
    return out

